# revision 2
# baseline (speedup 1.0000x reference)
import sys
from contextlib import ExitStack

import numpy as np
import ml_dtypes

sys.path.insert(0, "/opt/trn_rl_repo")

import concourse.bass as bass
import concourse.tile as tile
from concourse import bacc, mybir
from concourse.bass_utils import run_bass_kernel_spmd

B, H, W, CH = 4, 80, 80, 256
NCLS, DIM = 22, 256
ROWS = 40            # rows per core
NPIX = ROWS * W      # 3200 output pixels per core
NT = (ROWS + 2) * W + 2   # 3362 strip positions (1 halo row each side + 1 elem pad)
NTILE = NPIX // 128  # 25 output tiles of 128 pixels
SELW = 9 * 128       # per-tile selp row width (k-major, pixel minor)
F32 = mybir.dt.float32
BF16 = mybir.dt.bfloat16
BF16NP = ml_dtypes.bfloat16


def _build_nc():
    nc = bacc.Bacc("TRN2", target_bir_lowering=False, debug=False,
                   enable_asserts=True, num_devices=8)
    xt_d = nc.dram_tensor("xt", [128, 2 * NT], BF16, kind="ExternalInput").ap()
    wt_d = nc.dram_tensor("wt", [128, 18 * DIM], BF16, kind="ExternalInput").ap()
    selt_d = nc.dram_tensor("selt", [1, NTILE * SELW], BF16,
                            kind="ExternalInput").ap()
    out_d = nc.dram_tensor("out", [NPIX, DIM], F32, kind="ExternalOutput").ap()

    with tile.TileContext(nc) as tc, ExitStack() as ctx:
        xp = ctx.enter_context(tc.tile_pool(name="xp", bufs=1))
        wp = ctx.enter_context(tc.tile_pool(name="wp", bufs=1))
        stp = ctx.enter_context(tc.tile_pool(name="stp", bufs=1))
        Sp = ctx.enter_context(tc.tile_pool(name="Sp", bufs=3))
        xtsp = ctx.enter_context(tc.tile_pool(name="xtsp", bufs=3))
        outp = ctx.enter_context(tc.tile_pool(name="outp", bufs=3))
        zp = ctx.enter_context(tc.tile_pool(name="zp", bufs=6, space="PSUM"))

        xt = xp.tile([128, 2 * NT], BF16)
        wt = wp.tile([128, 18 * DIM], BF16)
        selt = stp.tile([1, NTILE * SELW], BF16)

        nc.sync.dma_start(selt[:], selt_d[:])
        # x chunk 0 first (tile 0's multiply needs it), then weights, then rest
        bnds = [0, 850, 1700, 2550, NT]
        for h in range(2):
            nc.sync.dma_start(xt[:, h * NT:h * NT + bnds[1]],
                              xt_d[:, h * NT:h * NT + bnds[1]])
        for k in range(9):
            nc.sync.dma_start(wt[:, k * 512:(k + 1) * 512],
                              wt_d[:, k * 512:(k + 1) * 512])
        for ci in range(1, 4):
            for h in range(2):
                a, b = h * NT + bnds[ci], h * NT + bnds[ci + 1]
                nc.sync.dma_start(xt[:, a:b], xt_d[:, a:b])

        for j in range(NTILE):
            S = Sp.tile([128, SELW], BF16)
            nc.gpsimd.partition_broadcast(
                S[:], selt[0:1, j * SELW:(j + 1) * SELW])
            xts = xtsp.tile([128, 2 * SELW], BF16)
            xb = xt[:, 0:1]
            pstep = xb.ap[0][0]
            for h in range(2):
                g = bass.AP(xb.tensor, xb.offset + h * NT + j * 128,
                            [[pstep, 128], [80, 3], [1, 3], [1, 128]])
                nc.vector.tensor_mul(xts[:, h * SELW:(h + 1) * SELW], g, S[:])
            z = zp.tile([128, DIM], F32)
            for k in range(9):
                for h in range(2):
                    nc.tensor.matmul(
                        z[:],
                        xts[:, h * SELW + k * 128:h * SELW + (k + 1) * 128],
                        wt[:, (2 * k + h) * DIM:(2 * k + h + 1) * DIM],
                        start=(k == 0 and h == 0), stop=(k == 8 and h == 1))
            outt = outp.tile([128, DIM], F32)
            nc.scalar.copy(outt[:], z[:])
            nc.sync.dma_start(out_d[j * 128:(j + 1) * 128, :], outt[:])
    nc.compile()
    return nc


_NC_CACHE = None


def _get_nc():
    global _NC_CACHE
    if _NC_CACHE is None:
        _NC_CACHE = _build_nc()
    return _NC_CACHE


def _prep_core(x, seg_mask, core):
    b, r0 = core // 2, 40 * (core % 2)
    xp = np.pad(x[b], ((1, 1), (0, 0), (0, 0)))        # [82,80,256]
    strip = xp[r0:r0 + 42].reshape(42 * W, CH)
    sp = np.zeros((NT, CH), np.float32)
    sp[1:1 + 42 * W] = strip
    spT = sp.T
    xt = np.ascontiguousarray(
        np.concatenate([spT[:128], spT[128:]], axis=1)).astype(BF16NP)

    pads = np.pad(seg_mask[b], ((1, 1), (1, 1), (0, 0)))  # [82,82,22]
    mc = seg_mask[b][r0:r0 + 40]                          # [40,80,22]
    smax = mc.max(-1, keepdims=True)
    eq = (mc == smax).astype(np.float32)
    sel = np.empty((40, 80, 9), np.float32)
    for k in range(9):
        di, dj = k // 3 - 1, k % 3 - 1
        sel[..., k] = (eq * pads[r0 + 1 + di:r0 + 41 + di,
                                 1 + dj:81 + dj]).sum(-1)
    cnt = (sel != 0).astype(np.float32).sum(-1, keepdims=True)
    selp = sel * (9.0 / np.maximum(cnt, 1.0))
    # [NTILE, 9, 128]: k-major, pixel-in-tile minor
    selt = np.ascontiguousarray(
        selp.reshape(NTILE, 128, 9).transpose(0, 2, 1)
    ).astype(BF16NP).reshape(1, NTILE * SELW)
    return xt, selt


def _build_in_maps(x, seg_mask, conv_w):
    w9 = conv_w.reshape(CH, 9, DIM)
    # [128, 9, 2, 256]: per k, both ch halves adjacent
    wt = np.ascontiguousarray(
        np.stack([w9[:128], w9[128:]], axis=2).reshape(128, 18 * DIM)
    ).astype(BF16NP)

    in_maps = []
    for core in range(8):
        xt, selt = _prep_core(x, seg_mask, core)
        in_maps.append({"xt": xt, "wt": wt, "selt": selt})
    return in_maps


def kernel(x, seg_mask, conv_w):
    x = np.asarray(x, np.float32)
    seg_mask = np.asarray(seg_mask, np.float32)
    conv_w = np.asarray(conv_w, np.float32)

    in_maps = _build_in_maps(x, seg_mask, conv_w)
    nc = _get_nc()
    res = run_bass_kernel_spmd(nc, in_maps, core_ids=list(range(8)))

    out = np.empty((B, H, W, DIM), np.float32)
    for core in range(8):
        b, r0 = core // 2, 40 * (core % 2)
        out[b, r0:r0 + 40] = res.results[core]["out"].reshape(ROWS, W, DIM)
    return out



# revision 3
# speedup vs baseline: 1.0310x; 1.0310x over previous
import sys
from contextlib import ExitStack

import numpy as np
import ml_dtypes

sys.path.insert(0, "/opt/trn_rl_repo")

import concourse.bass as bass
import concourse.tile as tile
from concourse import bacc, mybir
from concourse.bass_utils import run_bass_kernel_spmd

B, H, W, CH = 4, 80, 80, 256
NCLS, DIM = 22, 256
ROWS = 40            # rows per core
NPIX = ROWS * W      # 3200 output pixels per core
NT = (ROWS + 2) * W + 2   # 3362 strip positions (1 halo row each side + 1 elem pad)
NTILE = NPIX // 128  # 25 output tiles of 128 pixels
F32 = mybir.dt.float32
BF16 = mybir.dt.bfloat16
BF16NP = ml_dtypes.bfloat16

# pixel tiles are processed in groups; conv weights stay stationary on the PE
# across a whole group (N=G*128 moving columns per LDWEIGHTS)
GROUPS = [4, 8, 8, 5]          # tiles per group (sum = 25)
G_T0 = [0, 4, 12, 20]          # first tile of each group
SEL_BASE = [t0 * 9 * 128 for t0 in G_T0]   # selt column base per group


def _ap(t, off, dims):
    # raw AP on a pool tile's backing tensor: partition dim + free dims
    base = t[:, 0:1]
    return bass.AP(base.tensor, base.offset + off,
                   [[base.ap[0][0], 128]] + dims)


def _build_nc():
    nc = bacc.Bacc("TRN2", target_bir_lowering=False, debug=False,
                   enable_asserts=True, num_devices=8)
    xt_d = nc.dram_tensor("xt", [128, 2 * NT], BF16, kind="ExternalInput").ap()
    wt_d = nc.dram_tensor("wt", [128, 36 * 128], BF16, kind="ExternalInput").ap()
    selt_d = nc.dram_tensor("selt", [1, NTILE * 9 * 128], BF16,
                            kind="ExternalInput").ap()
    out_d = nc.dram_tensor("out", [2 * 128, NPIX], BF16,
                           kind="ExternalOutput").ap()

    with tile.TileContext(nc) as tc, ExitStack() as ctx:
        xp = ctx.enter_context(tc.tile_pool(name="xp", bufs=1))
        wp = ctx.enter_context(tc.tile_pool(name="wp", bufs=1))
        sbp = ctx.enter_context(tc.tile_pool(name="sbp", bufs=1))
        xtsp = ctx.enter_context(tc.tile_pool(name="xtsp", bufs=2))
        outp = ctx.enter_context(tc.tile_pool(name="outp", bufs=2))
        zp = ctx.enter_context(tc.tile_pool(name="zp", bufs=2, space="PSUM"))

        xt = xp.tile([128, 2 * NT], BF16)
        wt = wp.tile([128, 36 * 128], BF16)
        S = sbp.tile([128, NTILE * 9 * 128], BF16)

        # --- input DMAs, ordered so group 0 can start ASAP ---
        # sel broadcast: replicate the [1, cols] row across 128 partitions,
        # group 0 first (scalar/Activation HWDGE queue)
        for gi, G in enumerate(GROUPS):
            b0, cols = SEL_BASE[gi], 9 * G * 128
            src = bass.AP(selt_d.tensor, b0, [[0, 128], [1, cols]])
            nc.scalar.dma_start(S[:, b0:b0 + cols], src)
        # x strip: first chunk of both halves, then weights, then the rest
        # (sync/SP HWDGE queue)
        bnds = [0, 810, 1840, 2850, NT]
        for h in range(2):
            nc.sync.dma_start(xt[:, h * NT:h * NT + bnds[1]],
                              xt_d[:, h * NT:h * NT + bnds[1]])
        for k in range(9):
            nc.sync.dma_start(wt[:, k * 512:(k + 1) * 512],
                              wt_d[:, k * 512:(k + 1) * 512])
        for ci in range(1, 4):
            for h in range(2):
                a, b = h * NT + bnds[ci], h * NT + bnds[ci + 1]
                nc.sync.dma_start(xt[:, a:b], xt_d[:, a:b])

        for gi, G in enumerate(GROUPS):
            t0 = G_T0[gi]
            gw = G * 128          # moving columns in this group
            # gated patches: xts[c, (2k+h)*gw + t*128 + p]
            #   = xt[c, h*NT + (t0+t)*128 + i*80 + j + p] * sel[k, pixel]
            # one op per (h, j); j fixed keeps the innermost run unit-stride
            # (4B-aligned for even j -> DVE 2x mode); h=1 ops go to gpsimd
            xts = xtsp.tile([128, 18 * gw], BF16)
            for h in range(2):
                eng = nc.vector if h == 0 else nc.gpsimd
                for j in range(3):
                    o = _ap(xts, (2 * j + h) * gw,
                            [[6 * gw, 3], [128, G], [1, 128]])
                    i1 = _ap(xt, h * NT + t0 * 128 + j,
                             [[80, 3], [128, G], [1, 128]])
                    i2 = _ap(S, SEL_BASE[gi] + j * gw,
                             [[3 * gw, 3], [128, G], [1, 128]])
                    eng.tensor_mul(o, i1, i2)

            # PE: stationary = w[k,h,dh] chunk, moving = gated patches.
            # z[d, dh*1024 + p], accumulated over (k, h) in PSUM.
            z = zp.tile([128, 2048], F32)
            for k in range(9):
                for h in range(2):
                    c = 2 * k + h
                    for dh in range(2):
                        wc = wt[:, (c * 2 + dh) * 128:(c * 2 + dh + 1) * 128]
                        for n0 in range(0, gw, 512):
                            n1 = min(n0 + 512, gw)
                            nc.tensor.matmul(
                                z[:, dh * 1024 + n0:dh * 1024 + n1],
                                wc,
                                xts[:, c * gw + n0:c * gw + n1],
                                start=(k == 0 and h == 0),
                                stop=(k == 8 and h == 1))

            # evacuate PSUM -> SBUF bf16, then contiguous DMA to DRAM [d, p]
            outt = outp.tile([128, 2 * gw], BF16)
            for dh in range(2):
                nc.scalar.copy(outt[:, dh * gw:(dh + 1) * gw],
                               z[:, dh * 1024:dh * 1024 + gw])
                nc.sync.dma_start(
                    out_d[dh * 128:(dh + 1) * 128, t0 * 128:t0 * 128 + gw],
                    outt[:, dh * gw:(dh + 1) * gw])
    nc.compile()
    return nc


_NC_CACHE = None


def _get_nc():
    global _NC_CACHE
    if _NC_CACHE is None:
        _NC_CACHE = _build_nc()
    return _NC_CACHE


def _prep_core(x, seg_mask, core):
    b, r0 = core // 2, 40 * (core % 2)
    xp = np.pad(x[b], ((1, 1), (0, 0), (0, 0)))        # [82,80,256]
    strip = xp[r0:r0 + 42].reshape(42 * W, CH)
    sp = np.zeros((NT, CH), np.float32)
    sp[1:1 + 42 * W] = strip
    spT = sp.T
    xt = np.ascontiguousarray(
        np.concatenate([spT[:128], spT[128:]], axis=1)).astype(BF16NP)

    pads = np.pad(seg_mask[b], ((1, 1), (1, 1), (0, 0)))  # [82,82,22]
    mc = seg_mask[b][r0:r0 + 40]                          # [40,80,22]
    smax = mc.max(-1, keepdims=True)
    eq = (mc == smax).astype(np.float32)
    sel = np.empty((40, 80, 9), np.float32)
    for k in range(9):
        di, dj = k // 3 - 1, k % 3 - 1
        sel[..., k] = (eq * pads[r0 + 1 + di:r0 + 41 + di,
                                 1 + dj:81 + dj]).sum(-1)
    cnt = (sel != 0).astype(np.float32).sum(-1, keepdims=True)
    selp = (sel * (9.0 / np.maximum(cnt, 1.0))).reshape(NPIX, 9)
    # group-major: col = SEL_BASE[g] + k*(G*128) + t*128 + p
    parts = []
    for gi, G in enumerate(GROUPS):
        t0 = G_T0[gi]
        blk = selp[t0 * 128:(t0 + G) * 128]              # [G*128, 9]
        parts.append(blk.reshape(G, 128, 9).transpose(2, 0, 1).reshape(-1))
    selt = np.concatenate(parts).astype(BF16NP).reshape(1, NTILE * 9 * 128)
    return xt, selt


def _build_in_maps(x, seg_mask, conv_w):
    w9 = conv_w.reshape(2, 128, 9, DIM)                  # [h, c, k, d]
    # wt[c, ((2k+h)*2+dh)*128 + d] = w[h*128+c, k, dh*128+d]
    wt = np.ascontiguousarray(
        w9.reshape(2, 128, 9, 2, 128)                    # [h, c, k, dh, d]
          .transpose(1, 2, 0, 3, 4)                      # [c, k, h, dh, d]
          .reshape(128, 36 * 128)).astype(BF16NP)

    in_maps = []
    for core in range(8):
        xt, selt = _prep_core(x, seg_mask, core)
        in_maps.append({"xt": xt, "wt": wt, "selt": selt})
    return in_maps


def kernel(x, seg_mask, conv_w):
    x = np.asarray(x, np.float32)
    seg_mask = np.asarray(seg_mask, np.float32)
    conv_w = np.asarray(conv_w, np.float32)

    in_maps = _build_in_maps(x, seg_mask, conv_w)
    nc = _get_nc()
    res = run_bass_kernel_spmd(nc, in_maps, core_ids=list(range(8)))

    out = np.empty((B, H, W, DIM), np.float32)
    for core in range(8):
        b, r0 = core // 2, 40 * (core % 2)
        o = res.results[core]["out"].astype(np.float32)   # [256, 3200]
        out[b, r0:r0 + 40] = o.T.reshape(ROWS, W, DIM)
    return out


# revision 5
# speedup vs baseline: 1.0320x; 1.0009x over previous
import sys
from contextlib import ExitStack

import numpy as np
import ml_dtypes

sys.path.insert(0, "/opt/trn_rl_repo")

import concourse.bass as bass
import concourse.tile as tile
from concourse import bacc, mybir
from concourse.bass_utils import run_bass_kernel_spmd

B, H, W, CH = 4, 80, 80, 256
NCLS, DIM = 22, 256
ROWS = 40            # rows per core
NPIX = ROWS * W      # 3200 output pixels per core
NT = (ROWS + 2) * W + 2   # 3362 strip positions (1 halo row each side + 1 elem pad)
NTILE = NPIX // 128  # 25 output tiles of 128 pixels
F32 = mybir.dt.float32
BF16 = mybir.dt.bfloat16
BF16NP = ml_dtypes.bfloat16

# pixel tiles are processed in groups; conv weights stay stationary on the PE
# across a whole group (N=G*128 moving columns per LDWEIGHTS).
# patch taps are ordered kk = j*3 + i (j = column offset, i = row offset) so
# each gather op (fixed j) reads/writes contiguous kk blocks.
GROUPS = [4, 8, 8, 5]          # tiles per group (sum = 25)
G_T0 = [0, 4, 12, 20]          # first tile of each group
SEL_BASE = [t0 * 9 * 128 for t0 in G_T0]   # selt column base per group


def _ap(t, off, dims):
    # raw AP on a pool tile's backing tensor: partition dim + free dims
    base = t[:, 0:1]
    return bass.AP(base.tensor, base.offset + off,
                   [[base.ap[0][0], 128]] + dims)


def _build_nc():
    nc = bacc.Bacc("TRN2", target_bir_lowering=False, debug=False,
                   enable_asserts=True, num_devices=8)
    xt_d = nc.dram_tensor("xt", [128, 2 * NT], BF16, kind="ExternalInput").ap()
    wt_d = nc.dram_tensor("wt", [128, 36 * 128], BF16, kind="ExternalInput").ap()
    selt_d = nc.dram_tensor("selt", [1, NTILE * 9 * 128], BF16,
                            kind="ExternalInput").ap()
    out_d = nc.dram_tensor("out", [2 * 128, NPIX], BF16,
                           kind="ExternalOutput").ap()

    with tile.TileContext(nc) as tc, ExitStack() as ctx:
        xp = ctx.enter_context(tc.tile_pool(name="xp", bufs=1))
        wp = ctx.enter_context(tc.tile_pool(name="wp", bufs=1))
        sbp = ctx.enter_context(tc.tile_pool(name="sbp", bufs=1))
        xtsp = ctx.enter_context(tc.tile_pool(name="xtsp", bufs=2))
        outp = ctx.enter_context(tc.tile_pool(name="outp", bufs=2))
        zp = ctx.enter_context(tc.tile_pool(name="zp", bufs=2, space="PSUM"))

        xt = xp.tile([128, 2 * NT], BF16)
        wt = wp.tile([128, 36 * 128], BF16)
        S = sbp.tile([128, NTILE * 9 * 128], BF16)

        # --- input DMAs, ordered so group 0 can start ASAP ---
        # sel broadcast: replicate the [1, cols] row across 128 partitions,
        # per (group, j) chunk, group 0 first (scalar/Activation HWDGE queue)
        for gi, G in enumerate(GROUPS):
            for j in range(3):
                b0, cols = SEL_BASE[gi] + j * 3 * G * 128, 3 * G * 128
                src = bass.AP(selt_d.tensor, b0, [[0, 128], [1, cols]])
                nc.scalar.dma_start(S[:, b0:b0 + cols], src)
        # x strip: first chunk of both halves, then weights, then the rest
        # (sync/SP HWDGE queue)
        bnds = [0, 810, 1840, 2850, NT]
        for h in range(2):
            nc.sync.dma_start(xt[:, h * NT:h * NT + bnds[1]],
                              xt_d[:, h * NT:h * NT + bnds[1]])
        for k in range(9):
            nc.sync.dma_start(wt[:, k * 512:(k + 1) * 512],
                              wt_d[:, k * 512:(k + 1) * 512])
        for ci in range(1, 4):
            for h in range(2):
                a, b = h * NT + bnds[ci], h * NT + bnds[ci + 1]
                nc.sync.dma_start(xt[:, a:b], xt_d[:, a:b])

        for gi, G in enumerate(GROUPS):
            t0 = G_T0[gi]
            gw = G * 128          # moving columns in this group
            # gated patches, per h: xts_h[c, kk*gw + t*128 + p]
            #   = xt[c, h*NT + (t0+t)*128 + i*80 + j + p] * sel[kk, pixel]
            # one op per (h, j): out and sel are contiguous, only the xt
            # gather is strided; even j is 4B-aligned -> DVE 2x mode.
            # gpsimd (mode-agnostic) takes the misaligned j=1 ops.
            xts = [xtsp.tile([128, 9 * gw], BF16, name=f"xts{h}")
                   for h in range(2)]
            for j in (0, 1, 2):
                for h in range(2):
                    eng = nc.gpsimd if j == 1 else nc.vector
                    o = _ap(xts[h], j * 3 * gw, [[1, 3 * gw]])
                    i1 = _ap(xt, h * NT + t0 * 128 + j,
                             [[80, 3], [128, G], [1, 128]])
                    i2 = _ap(S, SEL_BASE[gi] + j * 3 * gw, [[1, 3 * gw]])
                    eng.tensor_mul(o, i1, i2)

            # PE: stationary = w[kk,h,dh] chunk, moving = gated patches.
            # z[d, dh*1024 + p], accumulated over (kk, h) in PSUM.
            z = zp.tile([128, 2048], F32)
            for kk in range(9):
                for h in range(2):
                    for dh in range(2):
                        wc = wt[:, ((kk * 2 + h) * 2 + dh) * 128:
                                ((kk * 2 + h) * 2 + dh + 1) * 128]
                        for n0 in range(0, gw, 512):
                            n1 = min(n0 + 512, gw)
                            nc.tensor.matmul(
                                z[:, dh * 1024 + n0:dh * 1024 + n1],
                                wc,
                                xts[h][:, kk * gw + n0:kk * gw + n1],
                                start=(kk == 0 and h == 0),
                                stop=(kk == 8 and h == 1))

            # evacuate PSUM -> SBUF bf16, then contiguous DMA to DRAM [d, p]
            outt = outp.tile([128, 2 * gw], BF16)
            for dh in range(2):
                nc.scalar.copy(outt[:, dh * gw:(dh + 1) * gw],
                               z[:, dh * 1024:dh * 1024 + gw])
                nc.scalar.dma_start(
                    out_d[dh * 128:(dh + 1) * 128, t0 * 128:t0 * 128 + gw],
                    outt[:, dh * gw:(dh + 1) * gw])
    nc.compile()
    return nc


_NC_CACHE = None


def _get_nc():
    global _NC_CACHE
    if _NC_CACHE is None:
        _NC_CACHE = _build_nc()
    return _NC_CACHE


def _prep_core(x, seg_mask, core):
    b, r0 = core // 2, 40 * (core % 2)
    xp = np.pad(x[b], ((1, 1), (0, 0), (0, 0)))        # [82,80,256]
    strip = xp[r0:r0 + 42].reshape(42 * W, CH)
    sp = np.zeros((NT, CH), np.float32)
    sp[1:1 + 42 * W] = strip
    spT = sp.T
    xt = np.ascontiguousarray(
        np.concatenate([spT[:128], spT[128:]], axis=1)).astype(BF16NP)

    pads = np.pad(seg_mask[b], ((1, 1), (1, 1), (0, 0)))  # [82,82,22]
    mc = seg_mask[b][r0:r0 + 40]                          # [40,80,22]
    smax = mc.max(-1, keepdims=True)
    eq = (mc == smax).astype(np.float32)
    sel = np.empty((40, 80, 9), np.float32)
    for k in range(9):
        di, dj = k // 3 - 1, k % 3 - 1
        sel[..., k] = (eq * pads[r0 + 1 + di:r0 + 41 + di,
                                 1 + dj:81 + dj]).sum(-1)
    cnt = (sel != 0).astype(np.float32).sum(-1, keepdims=True)
    selp = (sel * (9.0 / np.maximum(cnt, 1.0))).reshape(NPIX, 9)
    # group-major, kk = j*3+i ordered: col = SEL_BASE[g] + kk*(G*128) + t*128 + p
    KK2K = [(kk % 3) * 3 + kk // 3 for kk in range(9)]   # kk -> k = i*3+j
    parts = []
    for gi, G in enumerate(GROUPS):
        t0 = G_T0[gi]
        blk = selp[t0 * 128:(t0 + G) * 128][:, KK2K]     # [G*128, kk]
        parts.append(blk.reshape(G, 128, 9).transpose(2, 0, 1).reshape(-1))
    selt = np.concatenate(parts).astype(BF16NP).reshape(1, NTILE * 9 * 128)
    return xt, selt


def _build_in_maps(x, seg_mask, conv_w):
    w9 = conv_w.reshape(2, 128, 9, 2, 128)               # [h, c, k, dh, d]
    KK2K = [(kk % 3) * 3 + kk // 3 for kk in range(9)]
    # wt[c, ((kk*2+h)*2+dh)*128 + d] = w[h*128+c, KK2K[kk], dh*128+d]
    wt = np.ascontiguousarray(
        w9[:, :, KK2K]                                   # [h, c, kk, dh, d]
          .transpose(1, 2, 0, 3, 4)                      # [c, kk, h, dh, d]
          .reshape(128, 36 * 128)).astype(BF16NP)

    in_maps = []
    for core in range(8):
        xt, selt = _prep_core(x, seg_mask, core)
        in_maps.append({"xt": xt, "wt": wt, "selt": selt})
    return in_maps


def kernel(x, seg_mask, conv_w):
    x = np.asarray(x, np.float32)
    seg_mask = np.asarray(seg_mask, np.float32)
    conv_w = np.asarray(conv_w, np.float32)

    in_maps = _build_in_maps(x, seg_mask, conv_w)
    nc = _get_nc()
    res = run_bass_kernel_spmd(nc, in_maps, core_ids=list(range(8)))

    out = np.empty((B, H, W, DIM), np.float32)
    for core in range(8):
        b, r0 = core // 2, 40 * (core % 2)
        o = res.results[core]["out"].astype(np.float32)   # [256, 3200]
        out[b, r0:r0 + 40] = o.T.reshape(ROWS, W, DIM)
    return out


# revision 8
# speedup vs baseline: 1.2207x; 1.1829x over previous
import sys
from contextlib import ExitStack

import numpy as np
import ml_dtypes

sys.path.insert(0, "/opt/trn_rl_repo")

import concourse.bass as bass
import concourse.tile as tile
from concourse import bacc, mybir
from concourse.bass_utils import run_bass_kernel_spmd

B, H, W, CH = 4, 80, 80, 256
NCLS, DIM = 22, 256
ROWS = 40            # rows per core
NPIX = ROWS * W      # 3200 output pixels per core
NT = (ROWS + 2) * W + 2   # 3362 strip positions (1 halo row each side + 1 elem pad)
NTILE = NPIX // 128  # 25 output tiles of 128 pixels
F32 = mybir.dt.float32
BF16 = mybir.dt.bfloat16
BF16NP = ml_dtypes.bfloat16

# pixel tiles are processed in groups; conv weights stay stationary on the PE
# across a whole group (N=G*128 moving columns per LDWEIGHTS).
# patch taps are ordered kk = j*3 + i (j = column offset, i = row offset) so
# each gather op (fixed j) reads/writes contiguous kk blocks.
GROUPS = [4, 8, 8, 5]          # tiles per group (sum = 25)
G_T0 = [0, 4, 12, 20]          # first tile of each group
SEL_BASE = [t0 * 9 * 128 for t0 in G_T0]   # selt column base per group


def _ap(t, off, dims):
    # raw AP on a pool tile's backing tensor: partition dim + free dims
    base = t[:, 0:1]
    return bass.AP(base.tensor, base.offset + off,
                   [[base.ap[0][0], 128]] + dims)


def _build_nc():
    nc = bacc.Bacc("TRN2", target_bir_lowering=False, debug=False,
                   enable_asserts=True, num_devices=8)
    xt_d = nc.dram_tensor("xt", [128, 2 * NT], BF16, kind="ExternalInput").ap()
    wt_d = nc.dram_tensor("wt", [128, 36 * 128], BF16, kind="ExternalInput").ap()
    selt_d = nc.dram_tensor("selt", [1, NTILE * 9 * 128], BF16,
                            kind="ExternalInput").ap()
    out_d = nc.dram_tensor("out", [2 * 128, NPIX], BF16,
                           kind="ExternalOutput").ap()

    with tile.TileContext(nc) as tc, ExitStack() as ctx:
        xp = ctx.enter_context(tc.tile_pool(name="xp", bufs=1))
        wp = ctx.enter_context(tc.tile_pool(name="wp", bufs=1))
        sbp = ctx.enter_context(tc.tile_pool(name="sbp", bufs=1))
        xtsp = ctx.enter_context(tc.tile_pool(name="xtsp", bufs=2))
        outp = ctx.enter_context(tc.tile_pool(name="outp", bufs=2))
        zp = ctx.enter_context(tc.tile_pool(name="zp", bufs=2, space="PSUM"))

        xt = xp.tile([128, 2 * NT], BF16)
        xs = xp.tile([128, 2 * NT], BF16)   # xt shifted by one element
        wt = wp.tile([128, 36 * 128], BF16)
        S = sbp.tile([128, NTILE * 9 * 128], BF16)

        # --- input DMAs, ordered so group 0 can start ASAP ---
        # sel broadcast: replicate the [1, cols] row across 128 partitions,
        # per (group, j) chunk, group 0 first (scalar/Activation HWDGE queue)
        for gi, G in enumerate(GROUPS):
            for j in range(3):
                b0, cols = SEL_BASE[gi] + j * 3 * G * 128, 3 * G * 128
                src = bass.AP(selt_d.tensor, b0, [[0, 128], [1, cols]])
                nc.scalar.dma_start(S[:, b0:b0 + cols], src)
        # x strip: first chunk of both halves, then weights, then the rest
        # (sync/SP HWDGE queue)
        bnds = [0, 810, 1840, 2850, NT]
        for h in range(2):
            nc.sync.dma_start(xt[:, h * NT:h * NT + bnds[1]],
                              xt_d[:, h * NT:h * NT + bnds[1]])
        for h in range(2):
            # xs[c, pos] = xt[c, pos+1]: 4B-aligned base for the j=1 gather
            nc.sync.dma_start(xs[:, h * NT:h * NT + bnds[1]],
                              xt_d[:, h * NT + 1:h * NT + bnds[1] + 1])
        for k in range(9):
            nc.sync.dma_start(wt[:, k * 512:(k + 1) * 512],
                              wt_d[:, k * 512:(k + 1) * 512])
        for ci in range(1, 4):
            for h in range(2):
                a, b = h * NT + bnds[ci], h * NT + bnds[ci + 1]
                nc.sync.dma_start(xt[:, a:b], xt_d[:, a:b])
                e = min(b + 1, 2 * NT)
                nc.sync.dma_start(xs[:, a:e - 1], xt_d[:, a + 1:e])

        for gi, G in enumerate(GROUPS):
            t0 = G_T0[gi]
            gw = G * 128          # moving columns in this group
            # gated patches, per h: xts_h[c, kk*gw + t*128 + p]
            #   = xt[c, h*NT + (t0+t)*128 + i*80 + j + p] * sel[kk, pixel]
            # one op per (h, j): out and sel are contiguous, only the xt
            # gather is strided; even j is 4B-aligned -> DVE 2x mode.
            # gpsimd (mode-agnostic) takes the misaligned j=1 ops.
            xts = [xtsp.tile([128, 9 * gw], BF16, name=f"xts{h}")
                   for h in range(2)]
            for j in (0, 1, 2):
                for h in range(2):
                    # all on vector: DVE tensor_tensor and any gpsimd op
                    # fight for the same shared SBUF port (exclusive lock),
                    # so splitting across engines only adds blocking
                    src = xs if j == 1 else xt
                    o = _ap(xts[h], j * 3 * gw, [[1, 3 * gw]])
                    i1 = _ap(src, h * NT + t0 * 128 + (0 if j == 1 else j),
                             [[80, 3], [128, G], [1, 128]])
                    i2 = _ap(S, SEL_BASE[gi] + j * 3 * gw, [[1, 3 * gw]])
                    nc.vector.tensor_mul(o, i1, i2)

            # PE: stationary = w[kk,h,dh] chunk, moving = gated patches.
            # z[d, dh*1024 + p], accumulated over (kk, h) in PSUM.
            z = zp.tile([128, 2048], F32)
            for kk in range(9):
                for h in range(2):
                    for dh in range(2):
                        wc = wt[:, ((kk * 2 + h) * 2 + dh) * 128:
                                ((kk * 2 + h) * 2 + dh + 1) * 128]
                        for n0 in range(0, gw, 512):
                            n1 = min(n0 + 512, gw)
                            nc.tensor.matmul(
                                z[:, dh * 1024 + n0:dh * 1024 + n1],
                                wc,
                                xts[h][:, kk * gw + n0:kk * gw + n1],
                                start=(kk == 0 and h == 0),
                                stop=(kk == 8 and h == 1))

            # evacuate PSUM -> SBUF bf16, then contiguous DMA to DRAM [d, p]
            outt = outp.tile([128, 2 * gw], BF16)
            for dh in range(2):
                nc.scalar.copy(outt[:, dh * gw:(dh + 1) * gw],
                               z[:, dh * 1024:dh * 1024 + gw])
                nc.scalar.dma_start(
                    out_d[dh * 128:(dh + 1) * 128, t0 * 128:t0 * 128 + gw],
                    outt[:, dh * gw:(dh + 1) * gw])
    nc.compile()
    return nc


_NC_CACHE = None


def _get_nc():
    global _NC_CACHE
    if _NC_CACHE is None:
        _NC_CACHE = _build_nc()
    return _NC_CACHE


def _prep_core(x, seg_mask, core):
    b, r0 = core // 2, 40 * (core % 2)
    xp = np.pad(x[b], ((1, 1), (0, 0), (0, 0)))        # [82,80,256]
    strip = xp[r0:r0 + 42].reshape(42 * W, CH)
    sp = np.zeros((NT, CH), np.float32)
    sp[1:1 + 42 * W] = strip
    spT = sp.T
    xt = np.ascontiguousarray(
        np.concatenate([spT[:128], spT[128:]], axis=1)).astype(BF16NP)

    pads = np.pad(seg_mask[b], ((1, 1), (1, 1), (0, 0)))  # [82,82,22]
    mc = seg_mask[b][r0:r0 + 40]                          # [40,80,22]
    smax = mc.max(-1, keepdims=True)
    eq = (mc == smax).astype(np.float32)
    sel = np.empty((40, 80, 9), np.float32)
    for k in range(9):
        di, dj = k // 3 - 1, k % 3 - 1
        sel[..., k] = (eq * pads[r0 + 1 + di:r0 + 41 + di,
                                 1 + dj:81 + dj]).sum(-1)
    cnt = (sel != 0).astype(np.float32).sum(-1, keepdims=True)
    selp = (sel * (9.0 / np.maximum(cnt, 1.0))).reshape(NPIX, 9)
    # group-major, kk = j*3+i ordered: col = SEL_BASE[g] + kk*(G*128) + t*128 + p
    KK2K = [(kk % 3) * 3 + kk // 3 for kk in range(9)]   # kk -> k = i*3+j
    parts = []
    for gi, G in enumerate(GROUPS):
        t0 = G_T0[gi]
        blk = selp[t0 * 128:(t0 + G) * 128][:, KK2K]     # [G*128, kk]
        parts.append(blk.reshape(G, 128, 9).transpose(2, 0, 1).reshape(-1))
    selt = np.concatenate(parts).astype(BF16NP).reshape(1, NTILE * 9 * 128)
    return xt, selt


def _build_in_maps(x, seg_mask, conv_w):
    w9 = conv_w.reshape(2, 128, 9, 2, 128)               # [h, c, k, dh, d]
    KK2K = [(kk % 3) * 3 + kk // 3 for kk in range(9)]
    # wt[c, ((kk*2+h)*2+dh)*128 + d] = w[h*128+c, KK2K[kk], dh*128+d]
    wt = np.ascontiguousarray(
        w9[:, :, KK2K]                                   # [h, c, kk, dh, d]
          .transpose(1, 2, 0, 3, 4)                      # [c, kk, h, dh, d]
          .reshape(128, 36 * 128)).astype(BF16NP)

    in_maps = []
    for core in range(8):
        xt, selt = _prep_core(x, seg_mask, core)
        in_maps.append({"xt": xt, "wt": wt, "selt": selt})
    return in_maps


def kernel(x, seg_mask, conv_w):
    x = np.asarray(x, np.float32)
    seg_mask = np.asarray(seg_mask, np.float32)
    conv_w = np.asarray(conv_w, np.float32)

    in_maps = _build_in_maps(x, seg_mask, conv_w)
    nc = _get_nc()
    res = run_bass_kernel_spmd(nc, in_maps, core_ids=list(range(8)))

    out = np.empty((B, H, W, DIM), np.float32)
    for core in range(8):
        b, r0 = core // 2, 40 * (core % 2)
        o = res.results[core]["out"].astype(np.float32)   # [256, 3200]
        out[b, r0:r0 + 40] = o.T.reshape(ROWS, W, DIM)
    return out
